# revision 33
# baseline (speedup 1.0000x reference)
"""Trainium2 Bass kernel for masked cross-attention decoder.

Reference computation (per batch element b of B=1024):
  q = x[b] @ Wq.T                       (16, 512), split into 8 heads of 64
  k = l[b] @ Wk.T ; v = l[b] @ Wv.T     (128, 512)
  scores_h = q_h @ k_h.T / 8            masked to latents j <= (b % 128)
  attn = softmax(scores)                out = attn @ v
  y[b] = out @ Wo.T + bo                (16, 512)

Strategy: data-parallel over B across 8 cores (128 b per core; b % 128 spans
0..127 exactly once per core, so the masked work is identical on every core).
Only the first L = (b%128)+1 rows of l[b] are ever loaded or touched.

On-core pipeline (per core), mixed precision:
  - x.T via PE transposes (fp32), qT = Wq@x.T (fp32r matmuls, N=256)
  - qkT[c,(h,i)] = Wk-fold of q (fp32r, N=256), evacuated to bf16
  - per b: l cast-loaded to bf16 (SWDGE cast DMA, masked rows only),
    lT via PE transposes (bf16), scoresT[j,(h,i)] = lT.T @ qkT (bf16),
    exp on ACT, denominators via ones-matmul on PE (partition-dim sum),
    reciprocal on DVE, out_foldT[c,(h,i)] = l.T-weighted attn (bf16),
    normalization fused into the PSUM->SBUF evacuation multiply
  - pT = Wv-fold (bf16, N=256 over 16 b), y = p.T @ Wo.T + bias
    (fp32r, N=512 over 8 b), bias pre-broadcast host-side.
"""

import sys

for _p in ("/opt/trn_rl_repo", "/root/.axon_site/_ro/trn_rl_repo"):
    if _p not in sys.path:
        sys.path.append(_p)

import numpy as np
import ml_dtypes  # noqa: F401  (bf16 host-side if ever needed)

import concourse.bass as bass
import concourse.bacc as bacc
import concourse.mybir as mybir
import concourse.tile as tile
from concourse.bass_utils import run_bass_kernel_spmd

F32 = mybir.dt.float32
F32R = mybir.dt.float32r
BF16 = mybir.dt.bfloat16

DIM = 512
NT = 16          # tokens per batch element (DOWNSCALING)
NL = 128         # num latents
H = 8            # heads
DH = 64
N_CORES = 8
B_FULL = 1024
B_LOC = B_FULL // N_CORES   # 128 batch elements per core
BG = 16                      # batch-group size (free dim 256 = BG*NT)
CC = DIM // 128              # 4 contraction chunks

_PROGRAM_CACHE = {}


def _build_program(b_loc: int, reps: int = 1) -> bacc.Bacc:
    """Build the per-core Bass program. SPMD-uniform: identical for all cores.

    reps > 1 wraps the whole body in a device-side loop (for timing)."""
    nc = bacc.Bacc("TRN2", num_devices=N_CORES)
    n_grp = b_loc // BG

    x_d = nc.declare_dram_parameter("xs", [b_loc * NT, DIM], F32, isOutput=False)
    l_d = nc.declare_dram_parameter("ls", [b_loc, NL, DIM], F32, isOutput=False)
    wq_d = nc.declare_dram_parameter("wq", [DIM, DIM], F32, isOutput=False)
    wk_d = nc.declare_dram_parameter("wk", [DIM, DIM], F32, isOutput=False)
    wv_d = nc.declare_dram_parameter("wv", [DIM, DIM], F32, isOutput=False)
    wo_d = nc.declare_dram_parameter("wo", [DIM, DIM], F32, isOutput=False)
    bb_d = nc.declare_dram_parameter("biasb", [128, DIM], F32, isOutput=False)
    idf_d = nc.declare_dram_parameter("idf", [128, 128], F32, isOutput=False)
    idb_d = nc.declare_dram_parameter("idb", [128, 128], BF16, isOutput=False)
    on_d = nc.declare_dram_parameter("onesb", [128, 128], BF16, isOutput=False)
    y_d = nc.declare_dram_parameter("ys", [b_loc * NT, DIM], F32, isOutput=True)

    from contextlib import ExitStack

    with tile.TileContext(nc) as tc:
        with ExitStack() as _stk:
            ep = _stk.enter_context
            const = ep(tc.tile_pool(name="const", bufs=1))
            wtmp_pool = ep(tc.tile_pool(name="wtmp", bufs=2))
            xg_pool = ep(tc.tile_pool(name="xg", bufs=2))
            xt_pool = ep(tc.tile_pool(name="xt", bufs=2))
            qt_pool = ep(tc.tile_pool(name="qt", bufs=2))
            qkt_pool = ep(tc.tile_pool(name="qkt", bufs=2))
            lb_pool = ep(tc.tile_pool(name="lb", bufs=3))
            lt_pool = ep(tc.tile_pool(name="lt", bufs=6))
            exp_pool = ep(tc.tile_pool(name="expt", bufs=6))
            rcp_pool = ep(tc.tile_pool(name="rcp", bufs=6))
            oft_pool = ep(tc.tile_pool(name="oft", bufs=2))
            ptt_pool = ep(tc.tile_pool(name="ptt", bufs=2))
            yo_pool = ep(tc.tile_pool(name="yo", bufs=2))
            scr_pool = ep(tc.tile_pool(name="scr", bufs=3, space="DRAM"))
            # PSUM: 8 banks total; tiles are padded to one bank each.
            ps_t = ep(tc.tile_pool(name="ps_t", bufs=2, space="PSUM"))    # 2 banks
            ps_big = ep(tc.tile_pool(name="ps_big", bufs=2, space="PSUM"))  # 2 banks
            ps_att = ep(tc.tile_pool(name="ps_att", bufs=4, space="PSUM"))  # 4 banks
            ps_q = ps_qk = ps_p = ps_y = ps_big
            ps_sc = ps_dn = ps_of = ps_att
            # ---------------- constants ----------------
            idf = const.tile([128, 128], F32)
            nc.sync.dma_start(idf[:, :], idf_d[:, :])
            idb = const.tile([128, 128], BF16)
            nc.sync.dma_start(idb[:, :], idb_d[:, :])
            ones = const.tile([128, 128], BF16)
            nc.sync.dma_start(ones[:, :], on_d[:, :])
            biasb = const.tile([128, DIM], F32)
            nc.sync.dma_start(biasb[:, :], bb_d[:, :])

            # wk natural: [d = 128*s + p][c], 4 chunks stacked on a free axis
            wk_sb = const.tile([128, CC, DIM], F32R)
            for s in range(CC):
                wk_stage = wtmp_pool.tile([128, DIM], F32, tag="wtmp")
                nc.sync.dma_start(wk_stage[:, :], wk_d[128 * s:128 * (s + 1), :])
                nc.vector.tensor_copy(wk_sb[:, s, :], wk_stage[:, :])

            # Transposed weights via PE: src[d, c] natural -> dst[c, d]
            def build_transposed(src_d, dst, dst_dt):
                for s in range(CC):  # source row chunk (d)
                    wt = wtmp_pool.tile([128, DIM], F32, tag="wtmp")
                    nc.sync.dma_start(wt[:, :], src_d[128 * s:128 * (s + 1), :])
                    ps = ps_t.tile([128, CC, 128], F32, tag="ps_tr")
                    for t in range(CC):  # source col chunk (c)
                        nc.tensor.transpose(ps[:, t, :], wt[:, 128 * t:128 * (t + 1)], idf[:, :])
                        eng = nc.vector if (s + t) % 2 == 0 else nc.scalar
                        if eng is nc.vector:
                            nc.vector.tensor_copy(dst[:, t, 128 * s:128 * (s + 1)], ps[:, t, :])
                        else:
                            nc.scalar.copy(dst[:, t, 128 * s:128 * (s + 1)], ps[:, t, :])

            wqT = const.tile([128, CC, DIM], F32R)   # [c][d]
            build_transposed(wq_d, wqT, F32)
            wvT = const.tile([128, CC, DIM], BF16)  # [c][d]
            build_transposed(wv_d, wvT, BF16)
            woT = const.tile([128, CC, DIM], F32R)   # [d][e]
            build_transposed(wo_d, woT, F32)

            # ---------------- main loop ----------------
            def emit_group(g):
                # x.T for this group: xT[c, (b,i)] with 256 cols
                xT = xt_pool.tile([128, CC, 2 * 128], F32R)
                for xi in range(2):
                    xg = xg_pool.tile([128, DIM], F32, tag="xg")
                    r0 = g * 2 * 128 + xi * 128
                    nc.sync.dma_start(xg[:, :], x_d[r0:r0 + 128, :])
                    ps = ps_t.tile([128, CC, 128], F32, tag="ps_tr")
                    for t in range(CC):
                        nc.tensor.transpose(ps[:, t, :], xg[:, 128 * t:128 * (t + 1)], idf[:, :])
                    if xi % 2 == 0:
                        nc.vector.tensor_copy(xT[:, :, 128 * xi:128 * (xi + 1)], ps[:, :, :])
                    else:
                        nc.scalar.copy(xT[:, :, 128 * xi:128 * (xi + 1)], ps[:, :, :])

                # qT[d, (b,i)] = sum_c WqT[c, d]^T-style accumulation (fp32r)
                qT = qt_pool.tile([128, CC, 256], F32R)
                for u in range(CC):
                    qps = ps_q.tile([128, 256], F32, tag="ps_big")
                    for t in range(CC):
                        nc.tensor.matmul(
                            qps[:, :],
                            lhsT=wqT[:, t, 128 * u:128 * (u + 1)],
                            rhs=xT[:, t, :],
                            start=(t == 0), stop=(t == CC - 1),
                        )
                    if u % 2 == 0:
                        nc.vector.tensor_copy(qT[:, u, :], qps[:, :])
                    else:
                        nc.scalar.copy(qT[:, u, :], qps[:, :])

                # qkT[c', (b,h,i)] (bf16 out), via Wk natural slices (fp32r)
                # QKT layout: [c' = 128t+p][b][h][i]
                qkT = qkt_pool.tile([128, CC, BG, H, NT], BF16)
                for t in range(CC):
                    for h in range(H):
                        qkps = ps_qk.tile([128, 256], F32, tag="ps_big")
                        po = (h % 2) * 64
                        nc.tensor.matmul(
                            qkps[:, :],
                            lhsT=wk_sb[po:po + 64, h // 2, 128 * t:128 * (t + 1)],
                            rhs=qT[po:po + 64, h // 2, :],
                            start=True, stop=True,
                        )
                        src = qkps[:, :].rearrange("p (b i) -> p b i", b=BG)
                        if (t * H + h) % 4 == 0:
                            nc.vector.tensor_copy(qkT[:, t, :, h, :], src)
                        else:
                            nc.scalar.copy(qkT[:, t, :, h, :], src)

                # out_foldT accumulator for the group: [c][h][b][i] bf16
                oft = oft_pool.tile([128, CC, H, BG, NT], BF16)
                # reciprocal denominators, replicated on all partitions
                rcg = rcp_pool.tile([128, BG, H, NT], F32, tag="rcg")

                for bq in range(BG // 4):
                  lb4 = lb_pool.tile([128, 4, DIM], BF16, tag="lb")
                  m_hi = g * BG + 4 * bq + 3
                  L4 = m_hi + 1
                  # one masked cast-DMA covering 4 batch elements:
                  # dst[j, bb, c] = l[m0+bb, j, c], j < L4 (rectangular cover)
                  nc.gpsimd.dma_start(
                      lb4[:L4, :, :],
                      l_d[g * BG + 4 * bq:g * BG + 4 * bq + 4, :L4, :]
                      .rearrange("b j c -> j b c"))
                  scr4 = scr_pool.tile([4, NL, DIM], BF16, tag="scr")
                  nc.sync.dma_start(
                      scr4[:, :L4, :].rearrange("b j c -> j b c"),
                      lb4[:L4, :, :])
                  for bb in range(4):
                    bl = 4 * bq + bb
                    m = g * BG + bl       # local batch index == b % 128
                    L = m + 1             # allowed latents
                    lb = lb4[:, bb, :]

                    # lT[p, t, j] = l[j, 128t+p] via hardware xbar transpose
                    Lp = min(128, (L + 15) // 16 * 16)
                    lT = lt_pool.tile([128, CC, 128], BF16, tag="lt")
                    nc.sync.dma_start_transpose(
                        lT[:, :, :Lp], scr4[bb, :Lp, :])

                    # scoresT[j, (h,i)] accumulation over c chunks (bf16);
                    # denominators share the same PSUM bank (slice 1)
                    scdn = ps_sc.tile([128, 2, 128], F32, tag="ps_att")
                    sc = scdn[:, 0, :]
                    dn = scdn[:, 1, :]
                    for t in range(CC):
                        nc.tensor.matmul(
                            sc[:L, :],
                            lhsT=lT[:, t, :L],
                            rhs=qkT[:, t, bl, :, :].rearrange("p h i -> p (h i)"),
                            start=(t == 0), stop=(t == CC - 1),
                        )

                    # exp with 1/sqrt(dh) scale; bf16 out
                    expT = exp_pool.tile([128, 128], BF16, tag="expt")
                    nc.scalar.activation(
                        expT[:L, :], sc[:L, :],
                        mybir.ActivationFunctionType.Exp, scale=0.125)

                    # denominators broadcast to all partitions via ones-matmul
                    nc.tensor.matmul(
                        dn[:, :], lhsT=ones[:L, :], rhs=expT[:L, :],
                        start=True, stop=True)

                    # out_foldT[c, (h,i)] = sum_j l[j,c] * expT[j,(h,i)]
                    # (unnormalized; normalization happens at the pT stage).
                    # All 4 c-chunks land in one PSUM bank (disjoint slices).
                    ofp = ps_of.tile([128, CC, 128], F32, tag="ps_att")
                    for t in range(CC):
                        nc.tensor.matmul(
                            ofp[:, t, :],
                            lhsT=lb[:L, 128 * t:128 * (t + 1)],
                            rhs=expT[:L, :],
                            start=True, stop=True)
                    nc.vector.reciprocal(
                        rcg[:, bl, :, :],
                        dn[:, :].rearrange("p (h i) -> p h i", h=H))
                    if bl % 2 == 0:
                        nc.scalar.copy(
                            oft[:, :, :, bl, :],
                            ofp[:, :, :].rearrange("p t (h i) -> p t h i", h=H))
                    else:
                        nc.vector.tensor_copy(
                            oft[:, :, :, bl, :],
                            ofp[:, :, :].rearrange("p t (h i) -> p t h i", h=H))

                # pT[dh, (b,i)] per head, accumulated over c chunks (bf16 in)
                # PTT layout: [d = 128u+p][ (b,i) 256 ]
                ptt = ptt_pool.tile([128, CC, 256], F32R)
                for h in range(H):
                    pps = ps_p.tile([64, 256], F32, tag="ps_big")
                    for t in range(CC):
                        nc.tensor.matmul(
                            pps[:, :],
                            lhsT=wvT[:, t, 64 * h:64 * (h + 1)],
                            rhs=oft[:, t, h, :, :].rearrange("p b i -> p (b i)"),
                            start=(t == 0), stop=(t == CC - 1),
                        )
                    po = (h % 2) * 64
                    nc.vector.tensor_tensor(
                        ptt[po:po + 64, h // 2, :].rearrange(
                            "p (b i) -> p b i", b=BG),
                        pps[:, :].rearrange("p (b i) -> p b i", b=BG),
                        rcg[0:64, :, h, :],
                        op=mybir.AluOpType.mult,
                    )

                # y = p.T @ Wo.T + bias, per 8-b half group (fp32r, N=512)
                for half in range(2):
                    yp = ps_y.tile([128, DIM], F32, tag="ps_big")
                    for u in range(CC):
                        nc.tensor.matmul(
                            yp[:, :],
                            lhsT=ptt[:, u, 128 * half:128 * (half + 1)],
                            rhs=woT[:, u, :],
                            start=(u == 0), stop=(u == CC - 1),
                        )
                    yo = yo_pool.tile([128, DIM], F32, tag="yo")
                    nc.vector.tensor_tensor(
                        yo[:, :], yp[:, :], biasb[:, :], op=mybir.AluOpType.add)
                    r0 = g * 256 + half * 128
                    nc.sync.dma_start(y_d[r0:r0 + 128, :], yo[:, :])

            if reps > 1:
                with tc.For_i(0, reps, 1):
                    for g in range(n_grp):
                        emit_group(g)
            else:
                for g in range(n_grp):
                    emit_group(g)

    nc.compile()
    return nc


def _get_program(b_loc: int, reps: int = 1) -> bacc.Bacc:
    key = (b_loc, reps)
    if key not in _PROGRAM_CACHE:
        _PROGRAM_CACHE[key] = _build_program(b_loc, reps)
    return _PROGRAM_CACHE[key]


def kernel(x, l, Wq, Wk, Wv, Wo, bo, num_heads=8, _reps=1):
    x = np.ascontiguousarray(np.asarray(x, dtype=np.float32))
    l = np.ascontiguousarray(np.asarray(l, dtype=np.float32))
    Wq = np.ascontiguousarray(np.asarray(Wq, dtype=np.float32))
    Wk = np.ascontiguousarray(np.asarray(Wk, dtype=np.float32))
    Wv = np.ascontiguousarray(np.asarray(Wv, dtype=np.float32))
    Wo = np.ascontiguousarray(np.asarray(Wo, dtype=np.float32))
    bo = np.asarray(bo, dtype=np.float32)

    B = x.shape[0]
    assert B == B_FULL and int(num_heads) == H

    nc = _get_program(B_LOC, _reps)

    biasb = np.broadcast_to(bo[None, :], (128, DIM)).copy()
    idf = np.eye(128, dtype=np.float32)
    idb = np.eye(128, dtype=ml_dtypes.bfloat16)
    onesb = np.ones((128, 128), dtype=ml_dtypes.bfloat16)

    in_maps = []
    for c in range(N_CORES):
        sl = slice(c * B_LOC, (c + 1) * B_LOC)
        in_maps.append({
            "xs": x[sl].reshape(B_LOC * NT, DIM),
            "ls": l[sl],
            "wq": Wq, "wk": Wk, "wv": Wv, "wo": Wo,
            "biasb": biasb, "idf": idf, "idb": idb, "onesb": onesb,
        })

    res = run_bass_kernel_spmd(nc, in_maps, list(range(N_CORES)))
    y = np.empty((B, NT, DIM), dtype=np.float32)
    for c in range(N_CORES):
        y[c * B_LOC:(c + 1) * B_LOC] = res.results[c]["ys"].reshape(B_LOC, NT, DIM)
    return y
